# revision 30
# baseline (speedup 1.0000x reference)
"""DeepONet-style neural operator forward pass on 8 TRN2 NeuronCores, v3.

Data parallel over batch (16384 rows/core), weights replicated.
v3 changes vs v2 (engine-balance rewrite; ACT was the bottleneck at
~10.4us/block of ~18.3us measured):
  - single merged broadcast DMA for the sensor-replicated features
    ([128,4,NB] in one transfer vs 4) and no DMAs on the ACT queue.
  - pos^2 precomputed host-side into stacT rows 18-20 (stac tile is
    [21,NB]; qsl rows 18-20 = 1).
  - state residual folded into the tail matmul via a 10*I f16 lhsT on
    stac rows 0-12 (+0.1 scale in the combT epilogue): the strm DMA and
    the row-major add disappear.
  - pw tail matmul uses DoubleRow.
  - Newton-rsqrt chains run on DVE (SBUF tensor_scalar steps get the
    2x_2p DVE perf mode); qsb copy doubles as a max(q,eps) guard.
  - etp mul is one 4-wide Pool op; elementwise rebalanced so
    ACT/DVE/Pool/PE all land ~6.5-7.1us/block (model).
"""

import numpy as np
import ml_dtypes

import concourse.bass as bass
import concourse.mybir as mybir
import concourse.tile as tile
from concourse import bacc

F32 = mybir.dt.float32
F16 = mybir.dt.float16
F8 = mybir.dt.float8e4
I32 = mybir.dt.int32
AF = mybir.ActivationFunctionType
ALU = mybir.AluOpType
AX = mybir.AxisListType
PM = mybir.MatmulPerfMode

SD = 13
AD = 4
J = SD + AD      # 17 per-sensor features
NS = 32
BIN = NS * J     # 544
H1, H2, H4, H8 = 1024, 512, 256, 128
B_FULL = 131072
N_CORES = 8
RPC = B_FULL // N_CORES
NB = 512
NF = 21          # lhsT rows for q/trunk/qnet (stac rows 0-20)
import os as _os
LOOKAHEAD = int(_os.environ.get("K3_LA", "3"))
L1_ACT = int(_os.environ.get("K3_L1ACT", "2"))    # L1 pair-relus on ACT
L2_ACT = int(_os.environ.get("K3_L2ACT", "2"))    # L2 relus on ACT
Q_ACT = int(_os.environ.get("K3_QACT", "1"))      # qnet relu on ACT
ETAIL_DVE = int(_os.environ.get("K3_ETDVE", "0"))  # etail mul on DVE
B1AHEAD = int(_os.environ.get("K3_B1A", "1"))
QUAT_NR = int(_os.environ.get("K3_QNR", "1"))
NODMA = int(_os.environ.get("K3_NODMA", "0"))  # timing diag: skip per-blk DMA
NP8 = ml_dtypes.float8_e4m3


def _const_specs():
    e = []  # blob8 (fp8): (name, parts, cols)
    for mo in range(8):
        e.append((f"w1p0_{mo}", 128, 256))
        e.append((f"w1p1_{mo}", 128, 256))
        e.append((f"w1t_{mo}", 33, 256))
    for mo in range(4):
        for pr in range(4):
            e.append((f"w2_{mo}_{pr}", 128, 256))
    for mo in range(2):
        for pr in range(2):
            e.append((f"w3_{mo}_{pr}", 128, 256))
    for mo in range(2):
        e.append((f"tw2_{mo}", 128, 256))
    e.append(("pw8", 128, 32))
    e.append(("qw28", 128, 16))
    e.append(("ones8", 1, 1024))
    for mo in range(2):
        e.append((f"tw2b_{mo}", 1, 256))

    h = []  # blob16 (f16)
    h.append(("qsl", NF, NS))
    h.append(("tw1a", NF, 256))
    h.append(("qw1a", NF, 128))
    for c in range(4):
        h.append((f"rep_{c}", 128, 128))
    h.append(("idr13", SD, 16))

    f = []  # blob32 (f32)
    f.append(("bb2t", 128, 4))
    f.append(("bb3t", 128, 2))
    f.append(("c13", SD, 1))
    f.append(("rw13", SD, 1))
    f.append(("id13", SD, SD))

    def offsets(specs):
        out, o = {}, 0
        for name, p, w in specs:
            out[name] = (o, p, w)
            o += w
        return out, o

    eo, ew = offsets(e)
    ho, hw = offsets(h)
    fo, fw = offsets(f)
    return eo, ew, ho, hw, fo, fw


C8, C8W, C16, C16W, C32, C32W = _const_specs()


def build_nc(rpc=RPC, repeats=1, loop_n=None):
    assert rpc % NB == 0
    nblk = rpc // NB
    nc = bacc.Bacc(trn_type="TRN2")

    def inp(name, shape, dt=F32):
        return nc.dram_tensor(name, shape, dt, kind="ExternalInput").ap()

    # block-tiled: row b*21+f = feature f of block b (f 0-20)
    stacT = inp("stacT", [NF * nblk, NB], F16)
    blob8 = inp("blob8", [128, C8W], F8)
    blob16 = inp("blob16", [128, C16W], F16)
    blob32 = inp("blob32", [128, C32W], F32)

    # [blk, p, c, d] packed as [nblk*128, 52]; host unscrambles to [rpc, 13]
    out = nc.dram_tensor("out", [nblk * 128, 4 * SD], F32,
                         kind="ExternalOutput").ap()

    with tile.TileContext(nc) as tc:
        if loop_n is not None:
            with tc.For_i(0, loop_n, 1):
                _body(tc, nblk, locals())
        else:
            for _rep in range(repeats):
                _body(tc, nblk, locals())
    nc.compile()
    return nc


def _body(tc, nblk, t):
    nc = tc.nc
    import contextlib
    stack = contextlib.ExitStack()
    consts = stack.enter_context(tc.tile_pool(name="consts", bufs=1))
    sb_in = stack.enter_context(tc.tile_pool(name="sb_in", bufs=1))
    sb_sm = stack.enter_context(tc.tile_pool(name="sb_sm", bufs=1))
    sb_act = stack.enter_context(tc.tile_pool(name="sb_act", bufs=1))
    sb_out = stack.enter_context(tc.tile_pool(name="sb_out", bufs=1))
    import os as _os2
    _pairb = int(_os2.environ.get("K3_PAIRB", "2"))
    _ab = int(_os2.environ.get("K3_AB", "2"))
    _bb = int(_os2.environ.get("K3_BB", "2"))
    ps_pair = stack.enter_context(tc.tile_pool(name="ps_pair", bufs=_pairb,
                                               space="PSUM"))
    ps_a = stack.enter_context(tc.tile_pool(name="ps_a", bufs=_ab,
                                            space="PSUM"))
    ps_b = stack.enter_context(tc.tile_pool(name="ps_b", bufs=_bb,
                                            space="PSUM"))

    blob8_sb = consts.tile([128, C8W], F8, name="blob8_sb", tag="blob8_sb")
    blob16_sb = consts.tile([128, C16W], F16, name="blob16_sb",
                            tag="blob16_sb")
    blob32_sb = consts.tile([128, C32W], F32, name="blob32_sb",
                            tag="blob32_sb")
    NCH = 6
    step = (C8W + NCH - 1) // NCH
    for i in range(NCH):
        a, b = i * step, min((i + 1) * step, C8W)
        nc.sync.dma_start(out=blob8_sb[:, a:b], in_=t["blob8"][:, a:b])
    nc.sync.dma_start(out=blob16_sb, in_=t["blob16"])
    nc.sync.dma_start(out=blob32_sb, in_=t["blob32"])

    def v8(name):
        o, p, w = C8[name]
        return blob8_sb[0:p, o:o + w]

    def v16(name):
        o, p, w = C16[name]
        return blob16_sb[0:p, o:o + w]

    def v32(name):
        o, p, w = C32[name]
        return blob32_sb[0:p, o:o + w]

    w1p = [[v8(f"w1p{pi}_{mo}").rearrange("p (k m) -> p k m", k=2)
            for pi in range(2)] for mo in range(8)]
    w1t = [v8(f"w1t_{mo}").rearrange("p (k m) -> p k m", k=2)
           for mo in range(8)]
    w2 = [[v8(f"w2_{mo}_{pr}").rearrange("p (k m) -> p k m", k=2)
           for pr in range(4)] for mo in range(4)]
    w3 = [[v8(f"w3_{mo}_{pr}").rearrange("p (k m) -> p k m", k=2)
           for pr in range(2)] for mo in range(2)]
    tw2 = [v8(f"tw2_{mo}").rearrange("p (k m) -> p k m", k=2)
           for mo in range(2)]
    pw8 = v8("pw8").rearrange("p (k m) -> p k m", k=2)
    qw28 = v8("qw28")
    ones8 = v8("ones8").rearrange("p (k m) -> p k m", k=2)
    tw2b = [v8(f"tw2b_{mo}").rearrange("p (k m) -> p k m", k=2)
            for mo in range(2)]
    qsl = v16("qsl")
    tw1a = v16("tw1a")
    qw1a = v16("qw1a")
    rep = [v16(f"rep_{c}") for c in range(4)]
    idr13 = v16("idr13")
    bb2t = v32("bb2t")
    bb3t = v32("bb3t")
    c13 = v32("c13")
    rw13 = v32("rw13")
    id13 = v32("id13")

    # persistent enc-tail tiles: row 32 = ones (bias row), set once
    etails = [consts.tile([33, NB], F8, name=f"etail{i}", tag=f"etail{i}")
              for i in range(LOOKAHEAD + 1)]
    for et in etails:
        nc.gpsimd.memset(et[32:33, :], 1.0)

    stacT_d, out = t["stacT"], t["out"]

    blkst = {}

    def stage_a(blk):
        r0 = blk * NB
        b21 = blk * NF
        # ---- loads: stac (features 0-20), merged srp broadcast, srt ----
        stac = sb_in.tile([NF, NB], F16, tag="stac", bufs=LOOKAHEAD + 1)
        nc.sync.dma_start(out=stac, in_=stacT_d[b21:b21 + NF, :])
        # one f16 broadcast DMA: partition (jloc, s), free (q, n); feature
        # jloc*4+q is row b21+jloc*4+q, so (q, n) is contiguous in DRAM
        srp = sb_in.tile([128, 4, NB], F16, tag="srp", bufs=LOOKAHEAD + 1)
        srt = sb_in.tile([NS, NB], F16, tag="srt", bufs=LOOKAHEAD + 1)
        if not (NODMA and blk >= LOOKAHEAD + 1):
            nc.sync.dma_start(
                out=srp.rearrange("p q n -> p (q n)"),
                in_=stacT_d[b21:b21 + 16, :]
                    .rearrange("(j q) (o n) -> j o (q n)", j=4, o=1)
                    .broadcast_to([4, NS, 4 * NB]))
            nc.sync.dma_start(
                out=srt,
                in_=stacT_d[b21 + 16:b21 + 17, :]
                    .rearrange("j (o n) -> j o n", o=1)
                    .broadcast_to([1, NS, NB]))

        # ---- packed q = dist^2 [128, 128]: 4 quadrant matmuls ----
        q_ps = ps_a.tile([128, 128], F32, tag="a_ps", bufs=2)
        for c in range(4):
            nc.tensor.matmul(q_ps[c * 32:(c + 1) * 32, :], qsl,
                             stac[:, c * 128:(c + 1) * 128],
                             start=True, stop=True, tile_position=(0, c * 32))
        # Newton rsqrt (1 iter) on DVE; qsb = max(q, eps) guards q<0
        qsb = sb_sm.tile([128, 128], F32, tag="qsb", bufs=2)
        nc.vector.tensor_scalar(out=qsb, in0=q_ps, scalar1=1e-6, scalar2=None,
                                op0=ALU.max)
        r = sb_sm.tile([128, 128], F32, tag="r", bufs=2)
        y = sb_sm.tile([128, 128], F32, tag="y", bufs=2)
        u = sb_sm.tile([128, 128], F32, tag="u", bufs=2)
        y16 = sb_sm.tile([128, 128], F16, tag="y16", bufs=2)
        nc.vector.tensor_scalar(
            out=r.bitcast(I32), in0=qsb.bitcast(I32), scalar1=1, scalar2=None,
            op0=ALU.arith_shift_right)
        nc.vector.tensor_scalar(
            out=r.bitcast(I32), in0=r.bitcast(I32), scalar1=-1,
            scalar2=0x5F3759DF, op0=ALU.mult, op1=ALU.add)
        nc.vector.tensor_mul(y, qsb, r)
        nc.vector.tensor_mul(u, y, r)
        nc.vector.tensor_scalar(out=u, in0=u, scalar1=-0.5, scalar2=1.5,
                                op0=ALU.mult, op1=ALU.add)
        nc.vector.tensor_mul(y16, y, u)
        # replicate packed sqrt(q) 32->128, then exp straight out of psum
        yr_ps = ps_a.tile([128, NB], F32, tag="a_ps", bufs=2)
        for c in range(4):
            nc.tensor.matmul(yr_ps[:, c * 128:(c + 1) * 128], rep[c], y16,
                             start=True, stop=True)
        w_rep = sb_sm.tile([128, NB], F16, tag="w_rep", bufs=3)
        nc.scalar.activation(out=w_rep, in_=yr_ps, func=AF.Exp, bias=0.0,
                             scale=-2.0)

        # ---- enc: et = srp * w_rep (Pool, 4-wide); etail on Pool/DVE ----
        et = sb_in.tile([128, 4, NB], F8, tag="etp", bufs=LOOKAHEAD + 1)
        nc.gpsimd.tensor_mul(
            et, srp,
            w_rep.rearrange("p (o n) -> p o n", o=1)
                 .broadcast_to([128, 4, NB]))
        etail = etails[blk % (LOOKAHEAD + 1)]
        eng = nc.vector if ETAIL_DVE else nc.gpsimd
        eng.tensor_mul(etail[0:NS, :], srt, w_rep[0:NS, :])
        blkst[blk] = dict(stac=stac, et=et, etail=etail)

    def stage_b1(blk):
        st = blkst[blk]
        et, etail = st["et"], st["etail"]
        etail_dr = etail.rearrange("p (o n) -> p o n", o=1) \
                        .broadcast_to([33, 2, NB])

        # ---- L1: 544(+bias) -> 1024, fp8 DR ----
        h1 = []
        for po in range(4):
            ps = ps_pair.tile([128, 2, NB], F32, tag="pair_ps", bufs=2)
            for pl in range(2):
                mo = po * 2 + pl
                nc.tensor.matmul(ps[:, pl, :], w1p[mo][0], et[:, 0:2, :],
                                 start=True, stop=False,
                                 perf_mode=PM.DoubleRow)
                nc.tensor.matmul(ps[:, pl, :], w1p[mo][1], et[:, 2:4, :],
                                 start=False, stop=False,
                                 perf_mode=PM.DoubleRow)
                nc.tensor.matmul(ps[:, pl, :], w1t[mo], etail_dr,
                                 start=False, stop=True,
                                 perf_mode=PM.DoubleRow)
            hm = sb_act.tile([128, 2, NB], F8, tag="h1",
                             bufs=8 if B1AHEAD else 6)
            if po < L1_ACT:
                nc.scalar.activation(out=hm, in_=ps, func=AF.Relu,
                                     bias=0.0, scale=1.0)
            else:
                nc.vector.tensor_scalar(out=hm, in0=ps, scalar1=0.0,
                                        scalar2=None, op0=ALU.max)
            h1.append(hm)
        st["h1"] = h1

    def stage_b2(blk):
        st = blkst[blk]
        stac, h1 = st["stac"], st["h1"]

        # ---- L2: 1024 -> 512 fp8 DR, relu (bias via act/ts ptr) ----
        h2 = []
        for po in range(2):
            hp = sb_act.tile([128, 2, NB], F8, tag="h2", bufs=3)
            for pl in range(2):
                mo = po * 2 + pl
                ps = ps_b.tile([128, NB], F32, tag="b_ps", bufs=2)
                for pr in range(4):
                    nc.tensor.matmul(ps, w2[mo][pr], h1[pr],
                                     start=(pr == 0), stop=(pr == 3),
                                     perf_mode=PM.DoubleRow)
                if mo < L2_ACT:
                    nc.scalar.activation(out=hp[:, pl, :], in_=ps,
                                         func=AF.Relu,
                                         bias=bb2t[:, mo:mo + 1],
                                         scale=1.0)
                else:
                    nc.vector.tensor_scalar(
                        out=hp[:, pl, :], in0=ps,
                        scalar1=bb2t[:, mo:mo + 1], scalar2=0.0,
                        op0=ALU.add, op1=ALU.max)
            h2.append(hp)

        # ---- trunk1: K=21 f16 (bias folded via ones row) ----
        tt = sb_act.tile([128, 2, NB], F8, tag="tt", bufs=2)
        tt_ps = ps_pair.tile([128, 2, NB], F32, tag="pair_ps", bufs=2)
        for mo in range(2):
            nc.tensor.matmul(tt_ps[:, mo, :],
                             tw1a[:, mo * 128:(mo + 1) * 128],
                             stac, start=True, stop=True)
        nc.scalar.activation(out=tt, in_=tt_ps, func=AF.Tanh, bias=0.0,
                             scale=1.0)

        # ---- trunk2: fp8 DR (bias via ones chunk) + merged tanh ----
        trunk = sb_act.tile([128, 2, NB], F16, tag="trunk", bufs=2)
        pst = ps_pair.tile([128, 2, NB], F32, tag="pair_ps", bufs=2)
        for mo in range(2):
            nc.tensor.matmul(pst[:, mo, :], tw2[mo], tt,
                             start=True, stop=False,
                             perf_mode=PM.DoubleRow)
            nc.tensor.matmul(pst[:, mo, :], tw2b[mo][:, 0, :],
                             ones8[:, 0, :], start=False, stop=True)
        nc.scalar.activation(out=trunk, in_=pst, func=AF.Tanh,
                             bias=0.0, scale=1.0)

        # ---- qnet: K=21 f16 (bias folded), relu -> bq fp8 ----
        ps = ps_b.tile([128, NB], F32, tag="b_ps", bufs=2)
        nc.tensor.matmul(ps, qw1a, stac, start=True, stop=True)
        bq = sb_act.tile([128, NB], F8, tag="bq", bufs=2)
        if Q_ACT:
            nc.scalar.activation(out=bq, in_=ps, func=AF.Relu, bias=0.0,
                                 scale=1.0)
        else:
            nc.vector.tensor_scalar(out=bq, in0=ps, scalar1=0.0,
                                    scalar2=None, op0=ALU.max)

        # ---- L3 + interaction: fp8 DR, (ps+bb3)*trunk -> inter fp8 ----
        inter = sb_act.tile([128, 2, NB], F8, tag="inter", bufs=2)
        for mo in range(2):
            ps = ps_b.tile([128, NB], F32, tag="b_ps", bufs=2)
            for pr in range(2):
                nc.tensor.matmul(ps, w3[mo][pr], h2[pr],
                                 start=(pr == 0), stop=(pr == 1),
                                 perf_mode=PM.DoubleRow)
            nc.vector.scalar_tensor_tensor(
                out=inter[:, mo, :], in0=ps, scalar=bb3t[:, mo:mo + 1],
                in1=trunk[:, mo, :], op0=ALU.add, op1=ALU.mult)

        # ---- tail: pw DR + qw2 + 10I*state -> combT = rw*ps + c13 ----
        # out padded to 16 partitions (DR lhsT free stride must be 16B)
        tail_ps = ps_b.tile([16, NB], F32, tag="b_ps", bufs=2)
        nc.tensor.matmul(tail_ps, pw8, inter, start=True, stop=False,
                         perf_mode=PM.DoubleRow)
        nc.tensor.matmul(tail_ps, qw28, bq, start=False, stop=False)
        nc.tensor.matmul(tail_ps, idr13, stac[0:SD, :],
                         start=False, stop=True)
        combT = sb_out.tile([SD, NB], F32, tag="combT", bufs=2)
        nc.vector.tensor_scalar(
            out=combT, in0=tail_ps[0:SD, :], scalar1=rw13[:, 0:1],
            scalar2=c13[:, 0:1], op0=ALU.mult, op1=ALU.add)
        blkst[blk]["combT"] = combT

    def stage_c(blk):
        r0 = blk * NB
        st = blkst.pop(blk)
        combT = st["combT"]
        # ---- back to row-major, quat normalize, store ----
        tr_ps = ps_b.tile([128, 4, SD], F32, tag="b_ps", bufs=2)
        for c in range(4):
            nc.tensor.transpose(tr_ps[:, c, :],
                                combT[:, c * 128:(c + 1) * 128], id13)
        nxt = sb_out.tile([128, 4, SD], F32, tag="nxt", bufs=2)
        nc.vector.tensor_copy(nxt, tr_ps)
        # quat norm chain entirely on Pool (tree-sum, no DVE reduce: keeps
        # stage_c off DVE's in-order queue)
        sq = sb_out.tile([128, 4, 4], F32, tag="sq", bufs=2)
        nc.gpsimd.tensor_mul(sq, nxt[:, :, 3:7], nxt[:, :, 3:7])
        s2 = sb_out.tile([128, 4, 2], F32, tag="s2", bufs=2)
        nc.gpsimd.tensor_add(s2, sq[:, :, 0:2], sq[:, :, 2:4])
        qn = sb_out.tile([128, 4], F32, tag="qn", bufs=2)
        nc.gpsimd.tensor_add(qn.rearrange("p (c o) -> p c o", o=1),
                             s2[:, :, 0:1], s2[:, :, 1:2])
        rq = sb_out.tile([128, 4], F32, tag="rq", bufs=2)
        uq = sb_out.tile([128, 4], F32, tag="uq", bufs=2)
        yq = sb_out.tile([128, 4], F32, tag="yq", bufs=2)
        nc.vector.tensor_scalar(
            out=rq.bitcast(I32), in0=qn.bitcast(I32), scalar1=1, scalar2=None,
            op0=ALU.arith_shift_right)
        nc.vector.tensor_scalar(
            out=rq.bitcast(I32), in0=rq.bitcast(I32), scalar1=-1,
            scalar2=0x5F3759DF, op0=ALU.mult, op1=ALU.add)
        for it in range(QUAT_NR):
            nc.gpsimd.tensor_mul(yq, qn, rq)
            nc.gpsimd.tensor_mul(uq, yq, rq)
            nc.gpsimd.tensor_scalar(out=uq, in0=uq, scalar1=-0.5, scalar2=1.5,
                                    op0=ALU.mult, op1=ALU.add)
            nc.gpsimd.tensor_mul(rq, rq, uq)
        outt = sb_out.tile([128, 4, SD], F32, tag="outt", bufs=2)
        nc.gpsimd.tensor_copy(outt, nxt)
        nc.gpsimd.tensor_mul(
            outt[:, :, 3:7], nxt[:, :, 3:7],
            rq.rearrange("p (c o) -> p c o", o=1).broadcast_to([128, 4, 4]))
        # out DMA on the Pool SWDGE queue: it directly follows the quat
        # chain that produced outt, so its wait is already satisfied and
        # it never head-of-line blocks a prefetch or activation queue
        out_dst = t["out"][blk * 128:(blk + 1) * 128, :] \
            .rearrange("p (c d) -> p c d", c=4)
        nc.gpsimd.dma_start(out=out_dst, in_=outt)

    for b in range(min(LOOKAHEAD, nblk)):
        stage_a(b)
    if B1AHEAD:
        stage_b1(0)
        for blk in range(nblk):
            if blk + 1 < nblk:
                stage_b1(blk + 1)
            stage_b2(blk)
            stage_c(blk)
            if blk + LOOKAHEAD < nblk:
                stage_a(blk + LOOKAHEAD)
    else:
        for blk in range(nblk):
            stage_b1(blk)
            stage_b2(blk)
            stage_c(blk)
            if blk + LOOKAHEAD < nblk:
                stage_a(blk + LOOKAHEAD)
    stack.close()


def _host_prep(inputs):
    """Weight permutation/packing into dtype-segregated const blobs."""
    f = lambda x: np.ascontiguousarray(np.asarray(x, dtype=np.float32))
    sl = f(inputs["sensor_locations"])            # [32, 3]

    c = {}
    # qsl [21, 32]: rows 0-2 = -2*s^T, row 17 = |s|^2, rows 18-20 = 1
    qsl = np.zeros((NF, NS), np.float32)
    qsl[0:3, :] = -2.0 * sl.T
    qsl[17, :] = np.square(sl).sum(1)
    qsl[18:21, :] = 1.0
    c["qsl"] = qsl

    # trunk1/qnet lhsT with bias folded at ones row (17)
    tw1a = np.zeros((NF, 256), np.float32)
    tw1a[0:3, :] = f(inputs["tw1"])
    tw1a[17, :] = f(inputs["tb1"])
    c["tw1a"] = tw1a
    qw1a = np.zeros((NF, 128), np.float32)
    qw1a[0:3, :] = f(inputs["qw1"])
    qw1a[17, :] = f(inputs["qb1"])
    c["qw1a"] = qw1a

    for cc in range(4):
        m = np.zeros((128, 128), np.float32)
        for p in range(128):
            m[cc * 32 + p % 32, p] = 1.0
        c[f"rep_{cc}"] = m

    # W1 permuted + paired. enc row r = j*32 + s <- original row s*17 + j
    w1 = f(inputs["bw1"])                          # [544, 1024]
    jj, ss = np.meshgrid(np.arange(J), np.arange(NS), indexing="ij")
    perm = (ss * J + jj).reshape(-1)               # enc row -> original row
    w1p = w1[perm, :]                              # [544, 1024] j-major rows
    bb1 = f(inputs["bb1"])
    for mo in range(8):
        wm = w1p[:, mo * 128:(mo + 1) * 128]       # [544, 128]
        # lhsT slot (partition jloc*32+s, ks) holds feature jloc*4+pi*2+ks
        # (matches the merged srp broadcast DMA layout)
        for pi in range(2):
            blkw = np.zeros((128, 2, 128), np.float32)
            for ks in range(2):
                for jl in range(4):
                    ft = jl * 4 + pi * 2 + ks
                    blkw[jl * 32:(jl + 1) * 32, ks, :] = \
                        wm[ft * 32:(ft + 1) * 32, :]
            c[f"w1p{pi}_{mo}"] = blkw.reshape(128, 256)
        tl = np.zeros((33, 2, 128), np.float32)
        tl[0:32, 0, :] = wm[512:544, :]
        tl[32, 0, :] = bb1[mo * 128:(mo + 1) * 128]
        c[f"w1t_{mo}"] = tl.reshape(33, 256)

    w2 = f(inputs["bw2"])
    for mo in range(4):
        for pr in range(4):
            blkw = np.zeros((128, 2, 128), np.float32)
            for ks in range(2):
                blkw[:, ks, :] = w2[(2 * pr + ks) * 128:(2 * pr + ks + 1) * 128,
                                    mo * 128:(mo + 1) * 128]
            c[f"w2_{mo}_{pr}"] = blkw.reshape(128, 256)
    w3 = f(inputs["bw3"])
    for mo in range(2):
        for pr in range(2):
            blkw = np.zeros((128, 2, 128), np.float32)
            for ks in range(2):
                blkw[:, ks, :] = w3[(2 * pr + ks) * 128:(2 * pr + ks + 1) * 128,
                                    mo * 128:(mo + 1) * 128]
            c[f"w3_{mo}_{pr}"] = blkw.reshape(128, 256)
    tw2 = f(inputs["tw2"])
    for mo in range(2):
        blkw = np.zeros((128, 2, 128), np.float32)
        for ks in range(2):
            blkw[:, ks, :] = tw2[ks * 128:(ks + 1) * 128,
                                 mo * 128:(mo + 1) * 128]
        c[f"tw2_{mo}"] = blkw.reshape(128, 256)
    pwa = f(inputs["pw"])
    blkw = np.zeros((128, 2, 16), np.float32)
    for ks in range(2):
        blkw[:, ks, 0:SD] = pwa[ks * 128:(ks + 1) * 128, :]
    c["pw8"] = blkw.reshape(128, 32)
    qw2p = np.zeros((128, 16), np.float32)
    qw2p[:, 0:SD] = f(inputs["qw2"])
    c["qw28"] = qw2p
    c["ones8"] = np.ones((1, 1024), np.float32)
    tb2 = f(inputs["tb2"])
    for mo in range(2):
        bc = np.zeros((1, 2, 128), np.float32)
        bc[0, 0, :] = tb2[mo * 128:(mo + 1) * 128]
        c[f"tw2b_{mo}"] = bc.reshape(1, 256)

    def tcol(b, nm):
        return np.ascontiguousarray(f(b).reshape(nm, 128).T)

    rw = np.float32(np.asarray(inputs["residual_weight"]))
    c["bb2t"] = tcol(inputs["bb2"], 4)
    c["bb3t"] = tcol(inputs["bb3"], 2)
    c["c13"] = (rw * (f(inputs["pb"]) + f(inputs["qb2"]))).reshape(SD, 1)
    c["rw13"] = np.full((SD, 1), rw, np.float32)
    c["id13"] = np.eye(SD, dtype=np.float32)
    # 10*I f16: folds state/rw into the tail matmul (rw=0.1 exactly inverts);
    # M padded to 16 to satisfy the DR 16B lhsT stride restriction
    idr = np.zeros((SD, 16), np.float32)
    idr[:, 0:SD] = np.eye(SD, dtype=np.float32) / rw
    c["idr13"] = idr

    blob8 = np.zeros((128, C8W), NP8)
    for name, (o, p, w) in C8.items():
        blob8[0:p, o:o + w] = c[name].astype(NP8)
    blob16 = np.zeros((128, C16W), np.float16)
    for name, (o, p, w) in C16.items():
        blob16[0:p, o:o + w] = c[name].astype(np.float16)
    blob32 = np.zeros((128, C32W), np.float32)
    for name, (o, p, w) in C32.items():
        blob32[0:p, o:o + w] = c[name]
    return dict(blob8=blob8, blob16=blob16, blob32=blob32)


def _host_stact(state, action):
    """Block-tiled [n//NB*21, NB] f16: block b rows b*21+f, features
    f: 0-12 state, 13-16 action, 17 ones, 18-20 pos^2."""
    n = state.shape[0]
    sT = np.zeros((NF, n), np.float16)
    sT[0:SD, :] = state.T.astype(np.float16)
    sT[SD:J, :] = action.T.astype(np.float16)
    sT[17, :] = 1.0
    p16 = state[:, 0:3].T.astype(np.float16)
    sT[18:21, :] = (p16.astype(np.float32) ** 2).astype(np.float16)
    nblk = n // NB
    tiled = sT.reshape(NF, nblk, NB).transpose(1, 0, 2).reshape(
        NF * nblk, NB)
    return np.ascontiguousarray(tiled)


def _unscramble(out_arr):
    """[nblk*128, 52] device layout -> [rpc, 13] row-major."""
    nblk = out_arr.shape[0] // 128
    return np.ascontiguousarray(
        out_arr.reshape(nblk, 128, 4, SD).transpose(0, 2, 1, 3)
               .reshape(nblk * NB, SD))


_NC_CACHE = {}


def _get_nc(rpc=RPC):
    if rpc not in _NC_CACHE:
        _NC_CACHE[rpc] = build_nc(rpc)
    return _NC_CACHE[rpc]


def kernel(**inputs):
    from concourse.bass_utils import run_bass_kernel_spmd

    nc = _get_nc()
    common = _host_prep(inputs)
    state = np.ascontiguousarray(np.asarray(inputs["state"], np.float32))
    action = np.ascontiguousarray(np.asarray(inputs["action"], np.float32))
    in_maps = []
    for i in range(N_CORES):
        m = dict(common)
        m["stacT"] = _host_stact(state[i * RPC:(i + 1) * RPC],
                                 action[i * RPC:(i + 1) * RPC])
        in_maps.append(m)
    res = run_bass_kernel_spmd(nc, in_maps, list(range(N_CORES)))
    return np.concatenate([_unscramble(r["out"]) for r in res.results],
                          axis=0)


# revision 31
# speedup vs baseline: 1.0057x; 1.0057x over previous
"""DeepONet-style neural operator forward pass on 8 TRN2 NeuronCores, v3.

Data parallel over batch (16384 rows/core), weights replicated.
v3 changes vs v2 (engine-balance rewrite; ACT was the bottleneck at
~10.4us/block of ~18.3us measured):
  - single merged broadcast DMA for the sensor-replicated features
    ([128,4,NB] in one transfer vs 4) and no DMAs on the ACT queue.
  - pos^2 precomputed host-side into stacT rows 18-20 (stac tile is
    [21,NB]; qsl rows 18-20 = 1).
  - state residual folded into the tail matmul via a 10*I f16 lhsT on
    stac rows 0-12 (+0.1 scale in the combT epilogue): the strm DMA and
    the row-major add disappear.
  - pw tail matmul uses DoubleRow.
  - Newton-rsqrt chains run on DVE (SBUF tensor_scalar steps get the
    2x_2p DVE perf mode); qsb copy doubles as a max(q,eps) guard.
  - etp mul is one 4-wide Pool op; elementwise rebalanced so
    ACT/DVE/Pool/PE all land ~6.5-7.1us/block (model).
"""

import numpy as np
import ml_dtypes

import concourse.bass as bass
import concourse.mybir as mybir
import concourse.tile as tile
from concourse import bacc

F32 = mybir.dt.float32
F16 = mybir.dt.float16
F8 = mybir.dt.float8e4
I32 = mybir.dt.int32
AF = mybir.ActivationFunctionType
ALU = mybir.AluOpType
AX = mybir.AxisListType
PM = mybir.MatmulPerfMode

SD = 13
AD = 4
J = SD + AD      # 17 per-sensor features
NS = 32
BIN = NS * J     # 544
H1, H2, H4, H8 = 1024, 512, 256, 128
B_FULL = 131072
N_CORES = 8
RPC = B_FULL // N_CORES
NB = 512
NF = 21          # lhsT rows for q/trunk/qnet (stac rows 0-20)
import os as _os
LOOKAHEAD = int(_os.environ.get("K3_LA", "3"))
L1_ACT = int(_os.environ.get("K3_L1ACT", "2"))    # L1 pair-relus on ACT
L2_ACT = int(_os.environ.get("K3_L2ACT", "2"))    # L2 relus on ACT
Q_ACT = int(_os.environ.get("K3_QACT", "1"))      # qnet relu on ACT
ETAIL_DVE = int(_os.environ.get("K3_ETDVE", "0"))  # etail mul on DVE
B1AHEAD = int(_os.environ.get("K3_B1A", "1"))
QUAT_NR = int(_os.environ.get("K3_QNR", "1"))
NODMA = int(_os.environ.get("K3_NODMA", "0"))  # timing diag: skip per-blk DMA
NP8 = ml_dtypes.float8_e4m3


def _const_specs():
    e = []  # blob8 (fp8): (name, parts, cols)
    for mo in range(8):
        e.append((f"w1p0_{mo}", 128, 256))
        e.append((f"w1p1_{mo}", 128, 256))
        e.append((f"w1t_{mo}", 33, 256))
    for mo in range(4):
        for pr in range(4):
            e.append((f"w2_{mo}_{pr}", 128, 256))
    for mo in range(2):
        for pr in range(2):
            e.append((f"w3_{mo}_{pr}", 128, 256))
    for mo in range(2):
        e.append((f"tw2_{mo}", 128, 256))
    e.append(("pw8", 128, 32))
    e.append(("qw28", 128, 16))
    e.append(("ones8", 1, 1024))
    for mo in range(2):
        e.append((f"tw2b_{mo}", 1, 256))

    h = []  # blob16 (f16)
    h.append(("qsl", NF, NS))
    h.append(("tw1a", NF, 256))
    h.append(("qw1a", NF, 128))
    for c in range(4):
        h.append((f"rep_{c}", 128, 128))
    h.append(("idr13", SD, 16))

    f = []  # blob32 (f32)
    f.append(("bb2t", 128, 4))
    f.append(("bb3t", 128, 2))
    f.append(("c13", SD, 1))
    f.append(("rw13", SD, 1))
    f.append(("id13", SD, SD))

    def offsets(specs):
        out, o = {}, 0
        for name, p, w in specs:
            out[name] = (o, p, w)
            o += w
        return out, o

    eo, ew = offsets(e)
    ho, hw = offsets(h)
    fo, fw = offsets(f)
    return eo, ew, ho, hw, fo, fw


C8, C8W, C16, C16W, C32, C32W = _const_specs()


def build_nc(rpc=RPC, repeats=1, loop_n=None):
    assert rpc % NB == 0
    nblk = rpc // NB
    nc = bacc.Bacc(trn_type="TRN2")

    def inp(name, shape, dt=F32):
        return nc.dram_tensor(name, shape, dt, kind="ExternalInput").ap()

    # block-tiled: row b*21+f = feature f of block b (f 0-20)
    stacT = inp("stacT", [NF * nblk, NB], F16)
    blob8 = inp("blob8", [128, C8W], F8)
    blob16 = inp("blob16", [128, C16W], F16)
    blob32 = inp("blob32", [128, C32W], F32)

    # [blk, p, c, d] packed as [nblk*128, 52]; host unscrambles to [rpc, 13]
    out = nc.dram_tensor("out", [nblk * 128, 4 * SD], F32,
                         kind="ExternalOutput").ap()

    with tile.TileContext(nc) as tc:
        if loop_n is not None:
            with tc.For_i(0, loop_n, 1):
                _body(tc, nblk, locals())
        else:
            for _rep in range(repeats):
                _body(tc, nblk, locals())
    nc.compile()
    return nc


def _body(tc, nblk, t):
    nc = tc.nc
    import contextlib
    stack = contextlib.ExitStack()
    consts = stack.enter_context(tc.tile_pool(name="consts", bufs=1))
    sb_in = stack.enter_context(tc.tile_pool(name="sb_in", bufs=1))
    sb_sm = stack.enter_context(tc.tile_pool(name="sb_sm", bufs=1))
    sb_act = stack.enter_context(tc.tile_pool(name="sb_act", bufs=1))
    sb_out = stack.enter_context(tc.tile_pool(name="sb_out", bufs=1))
    import os as _os2
    _pairb = int(_os2.environ.get("K3_PAIRB", "2"))
    _ab = int(_os2.environ.get("K3_AB", "2"))
    _bb = int(_os2.environ.get("K3_BB", "2"))
    ps_pair = stack.enter_context(tc.tile_pool(name="ps_pair", bufs=_pairb,
                                               space="PSUM"))
    ps_a = stack.enter_context(tc.tile_pool(name="ps_a", bufs=_ab,
                                            space="PSUM"))
    ps_b = stack.enter_context(tc.tile_pool(name="ps_b", bufs=_bb,
                                            space="PSUM"))

    blob8_sb = consts.tile([128, C8W], F8, name="blob8_sb", tag="blob8_sb")
    blob16_sb = consts.tile([128, C16W], F16, name="blob16_sb",
                            tag="blob16_sb")
    blob32_sb = consts.tile([128, C32W], F32, name="blob32_sb",
                            tag="blob32_sb")
    NCH = 6
    step = (C8W + NCH - 1) // NCH
    for i in range(NCH):
        a, b = i * step, min((i + 1) * step, C8W)
        nc.sync.dma_start(out=blob8_sb[:, a:b], in_=t["blob8"][:, a:b])
    nc.sync.dma_start(out=blob16_sb, in_=t["blob16"])
    nc.sync.dma_start(out=blob32_sb, in_=t["blob32"])

    def v8(name):
        o, p, w = C8[name]
        return blob8_sb[0:p, o:o + w]

    def v16(name):
        o, p, w = C16[name]
        return blob16_sb[0:p, o:o + w]

    def v32(name):
        o, p, w = C32[name]
        return blob32_sb[0:p, o:o + w]

    w1p = [[v8(f"w1p{pi}_{mo}").rearrange("p (k m) -> p k m", k=2)
            for pi in range(2)] for mo in range(8)]
    w1t = [v8(f"w1t_{mo}").rearrange("p (k m) -> p k m", k=2)
           for mo in range(8)]
    w2 = [[v8(f"w2_{mo}_{pr}").rearrange("p (k m) -> p k m", k=2)
           for pr in range(4)] for mo in range(4)]
    w3 = [[v8(f"w3_{mo}_{pr}").rearrange("p (k m) -> p k m", k=2)
           for pr in range(2)] for mo in range(2)]
    tw2 = [v8(f"tw2_{mo}").rearrange("p (k m) -> p k m", k=2)
           for mo in range(2)]
    pw8 = v8("pw8").rearrange("p (k m) -> p k m", k=2)
    qw28 = v8("qw28")
    ones8 = v8("ones8").rearrange("p (k m) -> p k m", k=2)
    tw2b = [v8(f"tw2b_{mo}").rearrange("p (k m) -> p k m", k=2)
            for mo in range(2)]
    qsl = v16("qsl")
    tw1a = v16("tw1a")
    qw1a = v16("qw1a")
    rep = [v16(f"rep_{c}") for c in range(4)]
    idr13 = v16("idr13")
    bb2t = v32("bb2t")
    bb3t = v32("bb3t")
    c13 = v32("c13")
    rw13 = v32("rw13")
    id13 = v32("id13")

    # persistent enc-tail tiles: row 32 = ones (bias row), set once
    etails = [consts.tile([33, NB], F8, name=f"etail{i}", tag=f"etail{i}")
              for i in range(LOOKAHEAD + 1)]
    for et in etails:
        nc.gpsimd.memset(et[32:33, :], 1.0)

    stacT_d, out = t["stacT"], t["out"]

    blkst = {}

    def stage_a(blk):
        r0 = blk * NB
        b21 = blk * NF
        # ---- loads: stac (features 0-20), merged srp broadcast, srt ----
        stac = sb_in.tile([NF, NB], F16, tag="stac", bufs=LOOKAHEAD + 1)
        nc.sync.dma_start(out=stac, in_=stacT_d[b21:b21 + NF, :])
        # one f16 broadcast DMA: partition (jloc, s), free (q, n); feature
        # jloc*4+q is row b21+jloc*4+q, so (q, n) is contiguous in DRAM
        srp = sb_in.tile([128, 4, NB], F16, tag="srp", bufs=LOOKAHEAD + 1)
        srt = sb_in.tile([NS, NB], F16, tag="srt", bufs=LOOKAHEAD + 1)
        if not (NODMA and blk >= LOOKAHEAD + 1):
            nc.sync.dma_start(
                out=srp.rearrange("p q n -> p (q n)"),
                in_=stacT_d[b21:b21 + 16, :]
                    .rearrange("(j q) (o n) -> j o (q n)", j=4, o=1)
                    .broadcast_to([4, NS, 4 * NB]))
            nc.sync.dma_start(
                out=srt,
                in_=stacT_d[b21 + 16:b21 + 17, :]
                    .rearrange("j (o n) -> j o n", o=1)
                    .broadcast_to([1, NS, NB]))

        # ---- packed q = dist^2 [128, 128]: 4 quadrant matmuls ----
        q_ps = ps_a.tile([128, 128], F32, tag="a_ps", bufs=2)
        for c in range(4):
            nc.tensor.matmul(q_ps[c * 32:(c + 1) * 32, :], qsl,
                             stac[:, c * 128:(c + 1) * 128],
                             start=True, stop=True, tile_position=(0, c * 32))
        # Newton rsqrt (1 iter) on DVE; qsb = max(q, eps) guards q<0
        qsb = sb_sm.tile([128, 128], F32, tag="qsb", bufs=2)
        nc.vector.tensor_scalar(out=qsb, in0=q_ps, scalar1=1e-6, scalar2=None,
                                op0=ALU.max)
        r = sb_sm.tile([128, 128], F32, tag="r", bufs=2)
        y = sb_sm.tile([128, 128], F32, tag="y", bufs=2)
        u = sb_sm.tile([128, 128], F32, tag="u", bufs=2)
        y16 = sb_sm.tile([128, 128], F16, tag="y16", bufs=2)
        nc.vector.tensor_scalar(
            out=r.bitcast(I32), in0=qsb.bitcast(I32), scalar1=1, scalar2=None,
            op0=ALU.arith_shift_right)
        nc.vector.tensor_scalar(
            out=r.bitcast(I32), in0=r.bitcast(I32), scalar1=-1,
            scalar2=0x5F3759DF, op0=ALU.mult, op1=ALU.add)
        nc.vector.tensor_mul(y, qsb, r)
        nc.vector.tensor_mul(u, y, r)
        nc.vector.tensor_scalar(out=u, in0=u, scalar1=-0.5, scalar2=1.5,
                                op0=ALU.mult, op1=ALU.add)
        nc.vector.tensor_mul(y16, y, u)
        # replicate packed sqrt(q) 32->128, then exp straight out of psum
        yr_ps = ps_a.tile([128, NB], F32, tag="a_ps", bufs=2)
        for c in range(4):
            nc.tensor.matmul(yr_ps[:, c * 128:(c + 1) * 128], rep[c], y16,
                             start=True, stop=True)
        w_rep = sb_sm.tile([128, NB], F16, tag="w_rep", bufs=3)
        nc.scalar.activation(out=w_rep, in_=yr_ps, func=AF.Exp, bias=0.0,
                             scale=-2.0)

        # ---- enc: et = srp * w_rep (Pool, 4-wide); etail on Pool/DVE ----
        et = sb_in.tile([128, 4, NB], F8, tag="etp", bufs=LOOKAHEAD + 1)
        nc.gpsimd.tensor_mul(
            et, srp,
            w_rep.rearrange("p (o n) -> p o n", o=1)
                 .broadcast_to([128, 4, NB]))
        etail = etails[blk % (LOOKAHEAD + 1)]
        eng = nc.vector if ETAIL_DVE else nc.gpsimd
        eng.tensor_mul(etail[0:NS, :], srt, w_rep[0:NS, :])
        blkst[blk] = dict(stac=stac, et=et, etail=etail)

    def stage_b1(blk):
        st = blkst[blk]
        et, etail = st["et"], st["etail"]
        etail_dr = etail.rearrange("p (o n) -> p o n", o=1) \
                        .broadcast_to([33, 2, NB])

        # ---- L1: 544(+bias) -> 1024, fp8 DR ----
        h1 = []
        for po in range(4):
            ps = ps_pair.tile([128, 2, NB], F32, tag="pair_ps", bufs=2)
            for pl in range(2):
                mo = po * 2 + pl
                nc.tensor.matmul(ps[:, pl, :], w1p[mo][0], et[:, 0:2, :],
                                 start=True, stop=False,
                                 perf_mode=PM.DoubleRow)
                nc.tensor.matmul(ps[:, pl, :], w1p[mo][1], et[:, 2:4, :],
                                 start=False, stop=False,
                                 perf_mode=PM.DoubleRow)
                nc.tensor.matmul(ps[:, pl, :], w1t[mo], etail_dr,
                                 start=False, stop=True,
                                 perf_mode=PM.DoubleRow)
            hm = sb_act.tile([128, 2, NB], F8, tag="h1",
                             bufs=8 if B1AHEAD else 6)
            if po < L1_ACT:
                nc.scalar.activation(out=hm, in_=ps, func=AF.Relu,
                                     bias=0.0, scale=1.0)
            else:
                nc.vector.tensor_scalar(out=hm, in0=ps, scalar1=0.0,
                                        scalar2=None, op0=ALU.max)
            h1.append(hm)
        st["h1"] = h1

    def stage_b2(blk):
        st = blkst[blk]
        stac, h1 = st["stac"], st["h1"]

        # ---- L2: 1024 -> 512 fp8 DR, relu (bias via act/ts ptr) ----
        h2 = []
        for po in range(2):
            hp = sb_act.tile([128, 2, NB], F8, tag="h2", bufs=3)
            for pl in range(2):
                mo = po * 2 + pl
                ps = ps_b.tile([128, NB], F32, tag="b_ps", bufs=2)
                for pr in range(4):
                    nc.tensor.matmul(ps, w2[mo][pr], h1[pr],
                                     start=(pr == 0), stop=(pr == 3),
                                     perf_mode=PM.DoubleRow)
                if mo < L2_ACT:
                    nc.scalar.activation(out=hp[:, pl, :], in_=ps,
                                         func=AF.Relu,
                                         bias=bb2t[:, mo:mo + 1],
                                         scale=1.0)
                else:
                    nc.vector.tensor_scalar(
                        out=hp[:, pl, :], in0=ps,
                        scalar1=bb2t[:, mo:mo + 1], scalar2=0.0,
                        op0=ALU.add, op1=ALU.max)
            h2.append(hp)

        # ---- trunk1: K=21 f16 (bias folded via ones row) ----
        tt = sb_act.tile([128, 2, NB], F8, tag="tt", bufs=2)
        tt_ps = ps_pair.tile([128, 2, NB], F32, tag="pair_ps", bufs=2)
        for mo in range(2):
            nc.tensor.matmul(tt_ps[:, mo, :],
                             tw1a[:, mo * 128:(mo + 1) * 128],
                             stac, start=True, stop=True)
        nc.scalar.activation(out=tt, in_=tt_ps, func=AF.Tanh, bias=0.0,
                             scale=1.0)

        # ---- trunk2: fp8 DR (bias via ones chunk) + merged tanh ----
        trunk = sb_act.tile([128, 2, NB], F16, tag="trunk", bufs=2)
        pst = ps_pair.tile([128, 2, NB], F32, tag="pair_ps", bufs=2)
        for mo in range(2):
            nc.tensor.matmul(pst[:, mo, :], tw2[mo], tt,
                             start=True, stop=False,
                             perf_mode=PM.DoubleRow)
            nc.tensor.matmul(pst[:, mo, :], tw2b[mo][:, 0, :],
                             ones8[:, 0, :], start=False, stop=True)
        nc.scalar.activation(out=trunk, in_=pst, func=AF.Tanh,
                             bias=0.0, scale=1.0)

        # ---- qnet: K=21 f16 (bias folded), relu -> bq fp8 ----
        ps = ps_b.tile([128, NB], F32, tag="b_ps", bufs=2)
        nc.tensor.matmul(ps, qw1a, stac, start=True, stop=True)
        bq = sb_act.tile([128, NB], F8, tag="bq", bufs=2)
        if Q_ACT:
            nc.scalar.activation(out=bq, in_=ps, func=AF.Relu, bias=0.0,
                                 scale=1.0)
        else:
            nc.vector.tensor_scalar(out=bq, in0=ps, scalar1=0.0,
                                    scalar2=None, op0=ALU.max)

        # ---- L3 + interaction: fp8 DR, (ps+bb3)*trunk -> inter fp8 ----
        inter = sb_act.tile([128, 2, NB], F8, tag="inter", bufs=2)
        for mo in range(2):
            ps = ps_b.tile([128, NB], F32, tag="b_ps", bufs=2)
            for pr in range(2):
                nc.tensor.matmul(ps, w3[mo][pr], h2[pr],
                                 start=(pr == 0), stop=(pr == 1),
                                 perf_mode=PM.DoubleRow)
            nc.vector.scalar_tensor_tensor(
                out=inter[:, mo, :], in0=ps, scalar=bb3t[:, mo:mo + 1],
                in1=trunk[:, mo, :], op0=ALU.add, op1=ALU.mult)

        # ---- tail: pw DR + qw2 + 10I*state -> combT = rw*ps + c13 ----
        # out padded to 16 partitions (DR lhsT free stride must be 16B)
        tail_ps = ps_b.tile([16, NB], F32, tag="b_ps", bufs=2)
        nc.tensor.matmul(tail_ps, pw8, inter, start=True, stop=False,
                         perf_mode=PM.DoubleRow)
        nc.tensor.matmul(tail_ps, qw28, bq, start=False, stop=False)
        nc.tensor.matmul(tail_ps, idr13, stac[0:SD, :],
                         start=False, stop=True)
        combT = sb_out.tile([SD, NB], F32, tag="combT", bufs=2)
        nc.vector.tensor_scalar(
            out=combT, in0=tail_ps[0:SD, :], scalar1=rw13[:, 0:1],
            scalar2=c13[:, 0:1], op0=ALU.mult, op1=ALU.add)
        blkst[blk]["combT"] = combT

    def stage_c(blk):
        r0 = blk * NB
        st = blkst.pop(blk)
        combT = st["combT"]
        # ---- back to row-major, quat normalize, store ----
        tr_ps = ps_b.tile([128, 4, SD], F32, tag="b_ps", bufs=2)
        for c in range(4):
            nc.tensor.transpose(tr_ps[:, c, :],
                                combT[:, c * 128:(c + 1) * 128], id13)
        nxt = sb_out.tile([128, 4, SD], F32, tag="nxt", bufs=2)
        nc.vector.tensor_copy(nxt, tr_ps)
        # quat norm chain entirely on Pool (tree-sum, no DVE reduce: keeps
        # stage_c off DVE's in-order queue)
        sq = sb_out.tile([128, 4, 4], F32, tag="sq", bufs=2)
        nc.gpsimd.tensor_mul(sq, nxt[:, :, 3:7], nxt[:, :, 3:7])
        s2 = sb_out.tile([128, 4, 2], F32, tag="s2", bufs=2)
        nc.gpsimd.tensor_add(s2, sq[:, :, 0:2], sq[:, :, 2:4])
        qn = sb_out.tile([128, 4], F32, tag="qn", bufs=2)
        nc.gpsimd.tensor_add(qn.rearrange("p (c o) -> p c o", o=1),
                             s2[:, :, 0:1], s2[:, :, 1:2])
        rq = sb_out.tile([128, 4], F32, tag="rq", bufs=2)
        uq = sb_out.tile([128, 4], F32, tag="uq", bufs=2)
        yq = sb_out.tile([128, 4], F32, tag="yq", bufs=2)
        nc.vector.tensor_scalar(
            out=rq.bitcast(I32), in0=qn.bitcast(I32), scalar1=1, scalar2=None,
            op0=ALU.arith_shift_right)
        nc.vector.tensor_scalar(
            out=rq.bitcast(I32), in0=rq.bitcast(I32), scalar1=-1,
            scalar2=0x5F3759DF, op0=ALU.mult, op1=ALU.add)
        for it in range(QUAT_NR):
            nc.gpsimd.tensor_mul(yq, qn, rq)
            nc.gpsimd.tensor_mul(uq, yq, rq)
            nc.gpsimd.tensor_scalar(out=uq, in0=uq, scalar1=-0.5, scalar2=1.5,
                                    op0=ALU.mult, op1=ALU.add)
            nc.gpsimd.tensor_mul(rq, rq, uq)
        outt = sb_out.tile([128, 4, SD], F32, tag="outt", bufs=2)
        nc.gpsimd.tensor_copy(outt, nxt)
        nc.gpsimd.tensor_mul(
            outt[:, :, 3:7], nxt[:, :, 3:7],
            rq.rearrange("p (c o) -> p c o", o=1).broadcast_to([128, 4, 4]))
        out_dst = t["out"][blk * 128:(blk + 1) * 128, :] \
            .rearrange("p (c d) -> p c d", c=4)
        nc.sync.dma_start(out=out_dst, in_=outt)

    for b in range(min(LOOKAHEAD, nblk)):
        stage_a(b)
    if B1AHEAD:
        stage_b1(0)
        for blk in range(nblk):
            if blk + 1 < nblk:
                stage_b1(blk + 1)
            stage_b2(blk)
            stage_c(blk)
            if blk + LOOKAHEAD < nblk:
                stage_a(blk + LOOKAHEAD)
    else:
        for blk in range(nblk):
            stage_b1(blk)
            stage_b2(blk)
            stage_c(blk)
            if blk + LOOKAHEAD < nblk:
                stage_a(blk + LOOKAHEAD)
    stack.close()


def _host_prep(inputs):
    """Weight permutation/packing into dtype-segregated const blobs."""
    f = lambda x: np.ascontiguousarray(np.asarray(x, dtype=np.float32))
    sl = f(inputs["sensor_locations"])            # [32, 3]

    c = {}
    # qsl [21, 32]: rows 0-2 = -2*s^T, row 17 = |s|^2, rows 18-20 = 1
    qsl = np.zeros((NF, NS), np.float32)
    qsl[0:3, :] = -2.0 * sl.T
    qsl[17, :] = np.square(sl).sum(1)
    qsl[18:21, :] = 1.0
    c["qsl"] = qsl

    # trunk1/qnet lhsT with bias folded at ones row (17)
    tw1a = np.zeros((NF, 256), np.float32)
    tw1a[0:3, :] = f(inputs["tw1"])
    tw1a[17, :] = f(inputs["tb1"])
    c["tw1a"] = tw1a
    qw1a = np.zeros((NF, 128), np.float32)
    qw1a[0:3, :] = f(inputs["qw1"])
    qw1a[17, :] = f(inputs["qb1"])
    c["qw1a"] = qw1a

    for cc in range(4):
        m = np.zeros((128, 128), np.float32)
        for p in range(128):
            m[cc * 32 + p % 32, p] = 1.0
        c[f"rep_{cc}"] = m

    # W1 permuted + paired. enc row r = j*32 + s <- original row s*17 + j
    w1 = f(inputs["bw1"])                          # [544, 1024]
    jj, ss = np.meshgrid(np.arange(J), np.arange(NS), indexing="ij")
    perm = (ss * J + jj).reshape(-1)               # enc row -> original row
    w1p = w1[perm, :]                              # [544, 1024] j-major rows
    bb1 = f(inputs["bb1"])
    for mo in range(8):
        wm = w1p[:, mo * 128:(mo + 1) * 128]       # [544, 128]
        # lhsT slot (partition jloc*32+s, ks) holds feature jloc*4+pi*2+ks
        # (matches the merged srp broadcast DMA layout)
        for pi in range(2):
            blkw = np.zeros((128, 2, 128), np.float32)
            for ks in range(2):
                for jl in range(4):
                    ft = jl * 4 + pi * 2 + ks
                    blkw[jl * 32:(jl + 1) * 32, ks, :] = \
                        wm[ft * 32:(ft + 1) * 32, :]
            c[f"w1p{pi}_{mo}"] = blkw.reshape(128, 256)
        tl = np.zeros((33, 2, 128), np.float32)
        tl[0:32, 0, :] = wm[512:544, :]
        tl[32, 0, :] = bb1[mo * 128:(mo + 1) * 128]
        c[f"w1t_{mo}"] = tl.reshape(33, 256)

    w2 = f(inputs["bw2"])
    for mo in range(4):
        for pr in range(4):
            blkw = np.zeros((128, 2, 128), np.float32)
            for ks in range(2):
                blkw[:, ks, :] = w2[(2 * pr + ks) * 128:(2 * pr + ks + 1) * 128,
                                    mo * 128:(mo + 1) * 128]
            c[f"w2_{mo}_{pr}"] = blkw.reshape(128, 256)
    w3 = f(inputs["bw3"])
    for mo in range(2):
        for pr in range(2):
            blkw = np.zeros((128, 2, 128), np.float32)
            for ks in range(2):
                blkw[:, ks, :] = w3[(2 * pr + ks) * 128:(2 * pr + ks + 1) * 128,
                                    mo * 128:(mo + 1) * 128]
            c[f"w3_{mo}_{pr}"] = blkw.reshape(128, 256)
    tw2 = f(inputs["tw2"])
    for mo in range(2):
        blkw = np.zeros((128, 2, 128), np.float32)
        for ks in range(2):
            blkw[:, ks, :] = tw2[ks * 128:(ks + 1) * 128,
                                 mo * 128:(mo + 1) * 128]
        c[f"tw2_{mo}"] = blkw.reshape(128, 256)
    pwa = f(inputs["pw"])
    blkw = np.zeros((128, 2, 16), np.float32)
    for ks in range(2):
        blkw[:, ks, 0:SD] = pwa[ks * 128:(ks + 1) * 128, :]
    c["pw8"] = blkw.reshape(128, 32)
    qw2p = np.zeros((128, 16), np.float32)
    qw2p[:, 0:SD] = f(inputs["qw2"])
    c["qw28"] = qw2p
    c["ones8"] = np.ones((1, 1024), np.float32)
    tb2 = f(inputs["tb2"])
    for mo in range(2):
        bc = np.zeros((1, 2, 128), np.float32)
        bc[0, 0, :] = tb2[mo * 128:(mo + 1) * 128]
        c[f"tw2b_{mo}"] = bc.reshape(1, 256)

    def tcol(b, nm):
        return np.ascontiguousarray(f(b).reshape(nm, 128).T)

    rw = np.float32(np.asarray(inputs["residual_weight"]))
    c["bb2t"] = tcol(inputs["bb2"], 4)
    c["bb3t"] = tcol(inputs["bb3"], 2)
    c["c13"] = (rw * (f(inputs["pb"]) + f(inputs["qb2"]))).reshape(SD, 1)
    c["rw13"] = np.full((SD, 1), rw, np.float32)
    c["id13"] = np.eye(SD, dtype=np.float32)
    # 10*I f16: folds state/rw into the tail matmul (rw=0.1 exactly inverts);
    # M padded to 16 to satisfy the DR 16B lhsT stride restriction
    idr = np.zeros((SD, 16), np.float32)
    idr[:, 0:SD] = np.eye(SD, dtype=np.float32) / rw
    c["idr13"] = idr

    blob8 = np.zeros((128, C8W), NP8)
    for name, (o, p, w) in C8.items():
        blob8[0:p, o:o + w] = c[name].astype(NP8)
    blob16 = np.zeros((128, C16W), np.float16)
    for name, (o, p, w) in C16.items():
        blob16[0:p, o:o + w] = c[name].astype(np.float16)
    blob32 = np.zeros((128, C32W), np.float32)
    for name, (o, p, w) in C32.items():
        blob32[0:p, o:o + w] = c[name]
    return dict(blob8=blob8, blob16=blob16, blob32=blob32)


def _host_stact(state, action):
    """Block-tiled [n//NB*21, NB] f16: block b rows b*21+f, features
    f: 0-12 state, 13-16 action, 17 ones, 18-20 pos^2."""
    n = state.shape[0]
    sT = np.zeros((NF, n), np.float16)
    sT[0:SD, :] = state.T.astype(np.float16)
    sT[SD:J, :] = action.T.astype(np.float16)
    sT[17, :] = 1.0
    p16 = state[:, 0:3].T.astype(np.float16)
    sT[18:21, :] = (p16.astype(np.float32) ** 2).astype(np.float16)
    nblk = n // NB
    tiled = sT.reshape(NF, nblk, NB).transpose(1, 0, 2).reshape(
        NF * nblk, NB)
    return np.ascontiguousarray(tiled)


def _unscramble(out_arr):
    """[nblk*128, 52] device layout -> [rpc, 13] row-major."""
    nblk = out_arr.shape[0] // 128
    return np.ascontiguousarray(
        out_arr.reshape(nblk, 128, 4, SD).transpose(0, 2, 1, 3)
               .reshape(nblk * NB, SD))


_NC_CACHE = {}


def _get_nc(rpc=RPC):
    if rpc not in _NC_CACHE:
        _NC_CACHE[rpc] = build_nc(rpc)
    return _NC_CACHE[rpc]


def kernel(**inputs):
    from concourse.bass_utils import run_bass_kernel_spmd

    nc = _get_nc()
    common = _host_prep(inputs)
    state = np.ascontiguousarray(np.asarray(inputs["state"], np.float32))
    action = np.ascontiguousarray(np.asarray(inputs["action"], np.float32))
    in_maps = []
    for i in range(N_CORES):
        m = dict(common)
        m["stacT"] = _host_stact(state[i * RPC:(i + 1) * RPC],
                                 action[i * RPC:(i + 1) * RPC])
        in_maps.append(m)
    res = run_bass_kernel_spmd(nc, in_maps, list(range(N_CORES)))
    return np.concatenate([_unscramble(r["out"]) for r in res.results],
                          axis=0)
